# revision 28
# baseline (speedup 1.0000x reference)
"""Trainium2 Bass kernel for nn_AtomFeature (retrieval_knn).

Problem: B=2, N=4608 atoms, 3D coords. Outputs:
  atom_embedding (B,N,32)  - graph-normed tiled embedding table
  cross_dists    (B,N,32)  - distances to K=32 nearest neighbors
  edge_index     (B,N,32)  - indices of those neighbors

Sharding: the B*N = 9216 query rows are split across 8 cores (1152 rows
each; cores 0-3 handle batch 0, cores 4-7 batch 1). Each core receives
the full 4608 keys of its batch (replicated) - no collectives.

Architecture (v3): the otherwise-idle PE computes per-tile similarity
scores  score[q,j] = 2 q.k_j - |k_j|^2  ( = |q|^2 - d^2 up to a
row-constant) as 4-deep fp32 matmuls [4,128]^T @ [4,512] into PSUM,
evicted to SBUF in 1536-col blocks by ScalarE copies. The DVE then
extracts, per 384-col chunk, the top-8 scores (max8) and their local
indices (max_index) - just 24 short scans per 128-query tile, with all
12 max8s emitted before the 12 max_indexes so no instruction waits on
its producer's SBUF write-ack semaphore (measured ~0.6us/instr stall
otherwise). No match_replace, no on-device merge.

The host receives the 96 candidate indices per row, recomputes EXACT
f32 d^2 for them (reference rounding), and picks the top-32 by
(f32 dist, index) - exactly jax.lax.top_k's ordering including
equal-dist ties. Correctness never relies on the approximation:
 - every exact-top-32 member must be in its chunk's approximate top-8;
   a per-row completeness certificate checks that each chunk's weakest
   candidate is farther (by a margin >> the PE rounding error) than the
   selected 32nd neighbor, else the row is recomputed from scratch;
 - rows where equal approximate scores collapse two candidates into one
   index (max_index first-occurrence semantics) are detected by the
   duplicate check and likewise recomputed.
On this fixed seed-0 dataset the fallback hits ~100 of 9216 rows.
"""
import numpy as np

B = 2
N = 4608
D = 32
K = 32
NTYPES = 12
NCORES = 8
ROWS_PER_CORE = (B * N) // NCORES  # 1152
NTILES = ROWS_PER_CORE // 128      # 9
NQ = N // 4                        # 1152 quads (strip pairing)
NCH = 12                           # quad chunks per tile
CQ = NQ // NCH                     # 96 quads per chunk
NCAND = NCH * 8                    # 96 candidate quads per row
MMW = 512                          # matmul moving-dim block (PE limit)
BIG = 1000000.0
EPS_NORM = 1e-5
EPS_DIST = 1e-6
# completeness margin in d^2 units: must exceed 2x the worst-case PE
# score rounding error (~1.6e-2 here) plus the f32 sqrt tie window
CERT_MARGIN = 0.05

_compiled = None


def _build():
    import concourse.bacc as bacc
    from concourse import mybir
    from concourse.tile import TileContext

    f32 = mybir.dt.float32
    u16 = mybir.dt.uint16
    Alu = mybir.AluOpType
    Act = mybir.ActivationFunctionType

    f16 = mybir.dt.float16

    nc = bacc.Bacc(None, target_bir_lowering=False, debug=False)

    keys4h_ext = nc.declare_dram_parameter("keys4h", [4, N], f16, isOutput=False)
    keys4l_ext = nc.declare_dram_parameter("keys4l", [4, N], f16, isOutput=False)
    wqh_ext = nc.declare_dram_parameter("wqh", [4, ROWS_PER_CORE], f16, isOutput=False)
    wql_ext = nc.declare_dram_parameter("wql", [4, ROWS_PER_CORE], f16, isOutput=False)
    maskf_ext = nc.declare_dram_parameter("maskf", [1, N], f32, isOutput=False)
    embrep_ext = nc.declare_dram_parameter("embrep", [144, D], f32, isOutput=False)
    etabT_ext = nc.declare_dram_parameter("etabT", [D, NTYPES], f32, isOutput=False)
    scale_ext = nc.declare_dram_parameter("scalecol", [D, 1], f32, isOutput=False)
    shift_ext = nc.declare_dram_parameter("shiftcol", [D, 1], f32, isOutput=False)

    emb_out = nc.declare_dram_parameter("emb_out", [ROWS_PER_CORE, D], f32, isOutput=True)
    candl_out = nc.declare_dram_parameter("candl_out", [ROWS_PER_CORE, NCAND], u16, isOutput=True)

    arow_dram = nc.dram_tensor("arow_dram", [D, 2], f32)

    with TileContext(nc) as tc:
        with (
            tc.tile_pool(name="persist", bufs=1) as pp,
            tc.tile_pool(name="small", bufs=4) as sp,
            tc.psum_pool(name="psum", bufs=8) as qp,
        ):
            keys4h = pp.tile([4, N], f16)
            nc.sync.dma_start(out=keys4h[:, :], in_=keys4h_ext[:, :])
            keys4l = pp.tile([4, N], f16)
            nc.sync.dma_start(out=keys4l[:, :], in_=keys4l_ext[:, :])
            wqh = pp.tile([4, ROWS_PER_CORE], f16)
            nc.sync.dma_start(out=wqh[:, :], in_=wqh_ext[:, :])
            wql = pp.tile([4, ROWS_PER_CORE], f16)
            nc.sync.dma_start(out=wql[:, :], in_=wql_ext[:, :])

            ab = {}

            def stats_block():
                # ---- graph-norm statistics from per-type mask counts ----
                mf = pp.tile([1, N], f32)
                nc.sync.dma_start(out=mf[0:1, :], in_=maskf_ext[:, :])
                etabT = pp.tile([D, NTYPES], f32)
                nc.sync.dma_start(out=etabT[:, :], in_=etabT_ext[:, :])
                scol = pp.tile([D, 1], f32)
                nc.sync.dma_start(out=scol[:, :], in_=scale_ext[:, :])
                shcol = pp.tile([D, 1], f32)
                nc.sync.dma_start(out=shcol[:, :], in_=shift_ext[:, :])

                ts = pp.tile([1, NTYPES], f32)
                # mask[n], n = g*12 + r  ->  ts[r] = sum_g mask[g*12+r]
                nc.vector.reduce_sum(ts[:, :], mf[0:1, :].rearrange("p (g r) -> p r g", r=NTYPES),
                                     axis=mybir.AxisListType.X)
                cnt_raw = pp.tile([1, 1], f32)
                nc.vector.reduce_sum(cnt_raw[:, :], ts[:, :], axis=mybir.AxisListType.X)
                cnt1 = pp.tile([1, 1], f32)
                nc.vector.tensor_scalar_max(cnt1[:, :], cnt_raw[:, :], 1.0)
                rc = pp.tile([1, 1], f32)
                nc.vector.reciprocal(rc[:, :], cnt1[:, :])
                nmc = pp.tile([1, 1], f32)  # N - sum(mask)
                nc.vector.tensor_scalar(nmc[:, :], cnt_raw[:, :], -1.0, float(N), Alu.mult, Alu.add)

                tsb = pp.tile([D, NTYPES], f32)
                nc.gpsimd.partition_broadcast(tsb[:, :], ts[:, :])
                rcb = pp.tile([D, 1], f32)
                nc.gpsimd.partition_broadcast(rcb[:, :], rc[:, :])
                nmcb = pp.tile([D, 1], f32)
                nc.gpsimd.partition_broadcast(nmcb[:, :], nmc[:, :])

                tmp = pp.tile([D, NTYPES], f32)
                nc.vector.tensor_tensor(tmp[:, :], etabT[:, :], tsb[:, :], Alu.mult)
                meanT = pp.tile([D, 1], f32)
                nc.vector.reduce_sum(meanT[:, :], tmp[:, :], axis=mybir.AxisListType.X)
                nc.vector.tensor_scalar(meanT[:, :], meanT[:, :], rcb[:, 0:1], None, Alu.mult)
                negmeanT = pp.tile([D, 1], f32)
                nc.vector.tensor_scalar_mul(negmeanT[:, :], meanT[:, :], -1.0)

                sqT = pp.tile([D, NTYPES], f32)
                nc.scalar.activation(sqT[:, :], etabT[:, :], Act.Square, bias=negmeanT[:, 0:1], scale=1.0)
                nc.vector.tensor_tensor(sqT[:, :], sqT[:, :], tsb[:, :], Alu.mult)
                varT = pp.tile([D, 1], f32)
                nc.vector.reduce_sum(varT[:, :], sqT[:, :], axis=mybir.AxisListType.X)
                msq = pp.tile([D, 1], f32)
                nc.vector.tensor_tensor(msq[:, :], meanT[:, :], meanT[:, :], Alu.mult)
                nc.vector.tensor_scalar(msq[:, :], msq[:, :], nmcb[:, 0:1], None, Alu.mult)
                nc.vector.tensor_tensor(varT[:, :], varT[:, :], msq[:, :], Alu.add)
                nc.vector.tensor_scalar(varT[:, :], varT[:, :], rcb[:, 0:1], EPS_NORM, Alu.mult, Alu.add)

                # std = sqrt(varT) with 2 Newton refinements of the LUT sqrt
                stdT = pp.tile([D, 1], f32)
                nc.scalar.activation(stdT[:, :], varT[:, :], Act.Sqrt)
                for _ in range(2):
                    r_ = pp.tile([D, 1], f32, tag="newt_r")
                    nc.vector.reciprocal(r_[:, :], stdT[:, :])
                    nc.vector.tensor_tensor(r_[:, :], varT[:, :], r_[:, :], Alu.mult)
                    nc.vector.tensor_tensor(stdT[:, :], stdT[:, :], r_[:, :], Alu.add)
                    nc.vector.tensor_scalar_mul(stdT[:, :], stdT[:, :], 0.5)
                rstdT = pp.tile([D, 1], f32)
                nc.vector.reciprocal(rstdT[:, :], stdT[:, :])

                a0T = pp.tile([D, 1], f32)
                nc.vector.tensor_tensor(a0T[:, :], rstdT[:, :], scol[:, :], Alu.mult)
                a1T = pp.tile([D, 1], f32)
                nc.vector.tensor_tensor(a1T[:, :], meanT[:, :], a0T[:, :], Alu.mult)
                nc.vector.tensor_tensor(a1T[:, :], shcol[:, :], a1T[:, :], Alu.subtract)

                # (D,1) columns -> (1,D) rows via DRAM bounce, then broadcast
                nc.sync.dma_start(out=arow_dram[:, 0:1], in_=a0T[:, :])
                nc.sync.dma_start(out=arow_dram[:, 1:2], in_=a1T[:, :])
                a0row = pp.tile([1, D], f32)
                nc.sync.dma_start(out=a0row[:, :], in_=arow_dram[:, 0:1])
                a1row = pp.tile([1, D], f32)
                nc.sync.dma_start(out=a1row[:, :], in_=arow_dram[:, 1:2])
                a0full = pp.tile([128, D], f32)
                nc.gpsimd.partition_broadcast(a0full[:, :], a0row[:, :])
                a1full = pp.tile([128, D], f32)
                nc.gpsimd.partition_broadcast(a1full[:, :], a1row[:, :])
                ab["a0"] = a0full
                ab["a1"] = a1full

            # three persistent score planes: the PE/Act feed runs up to two
            # tiles ahead of the DVE scans, so the PE stream forms long
            # continuous bursts that ramp to the full p-state
            nd_p = [pp.tile([128, N], f32, name=f"nd{i}") for i in range(3)]
            # strip-quad max pre-reduction planes: quad q covers columns
            # {q, q+1152, q+2304, q+3456}
            m2 = pp.tile([128, N // 2], f32)
            m4 = pp.tile([128, NQ], f32)

            stats_block()

            def emb_block():
                # the embedding depends only on the (tiled) table and the
                # graph-norm affine, and the per-tile type offset cycles
                # through just {0, 8, 4}: three tiles cover all nine output
                # blocks (strided DRAM writes). Nothing embedding-related
                # runs in the per-tile loop; the host applies the mask.
                for i, off in enumerate((0, 8, 4)):
                    et = pp.tile([128, D], f32, name=f"et{i}")
                    nc.sync.dma_start(out=et[:, :], in_=embrep_ext[off:off + 128, :])
                    z = pp.tile([128, D], f32, name=f"z{i}")
                    nc.gpsimd.tensor_tensor(z[:, :], et[:, :], ab["a0"][:, :], Alu.mult)
                    nc.gpsimd.tensor_tensor(z[:, :], z[:, :], ab["a1"][:, :], Alu.add)
                    for t in range(i, NTILES, 3):
                        nc.sync.dma_start(out=emb_out[t * 128:(t + 1) * 128, :], in_=z[:, :])

            staged = {}

            def feed(t):
                lo = t * 128
                nd = nd_p[t % 3]
                wh = wqh[:, lo:lo + 128]
                wl = wql[:, lo:lo + 128]
                # split-fp16 scores: fp16 matmuls sustain ~115ns/512 cols
                # (18x the fp32 path). score = Wh@Xh + Wh@Xl + Wl@Xh
                # accumulated in PSUM; the dropped Wl@Xl term plus PSUM
                # rounding is < 5e-3, far inside CERT_MARGIN. Single-bank
                # PSUM tiles with a deep pool keep the PE stream rolling.
                for m in range(N // MMW):
                    s = m * MMW
                    ps = qp.tile([128, MMW], f32, name=f"ps{t}_{m}", tag="ps")
                    nc.tensor.matmul(ps[:, :], wh, keys4h[:, s:s + MMW], start=True, stop=False)
                    nc.tensor.matmul(ps[:, :], wh, keys4l[:, s:s + MMW], start=False, stop=False)
                    nc.tensor.matmul(ps[:, :], wl, keys4h[:, s:s + MMW], start=False, stop=True)
                    nc.scalar.copy(nd[:, s:s + MMW], ps[:, :])
                staged[t] = nd

            staged2 = {}

            def chunks(t):
                nd = staged.pop(t)
                # quad-max pre-reduction: gpsimd folds the halves, DVE folds
                # once more; the 24 short scans then cover only 1152 cols.
                # Exactness is preserved because the host refines all 4
                # members of every candidate quad, and the completeness
                # certificate bounds unseen quads by their quadmax.
                half = N // 2
                nc.vector.tensor_tensor(m2[:, :], nd[:, 0:half], nd[:, half:N], Alu.max)
                nc.vector.tensor_tensor(m4[:, :], m2[:, 0:NQ], m2[:, NQ:half], Alu.max)
                cand_v = sp.tile([128, NCAND], f32, name=f"cv{t}", tag="cv")
                candL = sp.tile([128, NCAND], u16, name=f"cl{t}", tag="cl")
                # all max8s first, then all max_indexes: by the time
                # max_index(c) issues, max8(c) retired 11 scans earlier and
                # its SBUF write-ack semaphore has long fired - no stall
                for c in range(NCH):
                    nc.vector.max(cand_v[:, 8 * c:8 * c + 8], m4[:, c * CQ:(c + 1) * CQ])
                for c in range(NCH):
                    nc.vector.max_index(candL[:, 8 * c:8 * c + 8],
                                        cand_v[:, 8 * c:8 * c + 8],
                                        m4[:, c * CQ:(c + 1) * CQ])
                staged2[t] = candL

            def tail(t):
                lo = t * 128
                candL = staged2.pop(t)
                nc.sync.dma_start(out=candl_out[lo:lo + 128, :], in_=candL[:, :])

            feed(0)
            feed(1)
            emb_block()
            for t in range(NTILES):
                chunks(t)
                if t + 2 < NTILES:
                    feed(t + 2)
                if t >= 1:
                    tail(t - 1)
            tail(NTILES - 1)

    nc.compile()
    return nc


def _get_compiled():
    global _compiled
    if _compiled is None:
        _compiled = _build()
    return _compiled


def _exact_d2_f32(q, kc):
    """Reference-rounding f32 squared distance: ((dx^2+dy^2)+dz^2)."""
    d = (q - kc).astype(np.float32)
    t = (d * d).astype(np.float32)
    return ((t[..., 0] + t[..., 1]).astype(np.float32) + t[..., 2]).astype(np.float32)


def build_in_maps(atom_coords, atom_mask, emb_table, scale, shift):
    atom_coords = np.asarray(atom_coords, dtype=np.float32)
    atom_mask = np.asarray(atom_mask, dtype=np.float32)
    emb_table = np.asarray(emb_table, dtype=np.float32)
    scale = np.asarray(scale, dtype=np.float32).reshape(D, 1)
    shift = np.asarray(shift, dtype=np.float32).reshape(D, 1)

    embrep = np.ascontiguousarray(np.tile(emb_table, (12, 1)))  # (144, D)
    etabT = np.ascontiguousarray(emb_table.T)                    # (D, 12)

    c64 = atom_coords.astype(np.float64)

    def f16_split(a32):
        hi = a32.astype(np.float16)
        lo = (a32 - hi.astype(np.float32)).astype(np.float16)
        return np.ascontiguousarray(hi), np.ascontiguousarray(lo)

    # keys4 rows: kx, ky, kz, -|k|^2 ; wq rows: 2qx, 2qy, 2qz, 1
    # each sent as an fp16 (hi, lo) pair for the split-fp16 matmuls
    keys4_b = []
    wq_b = []
    for b in range(B):
        k2 = -(c64[b] ** 2).sum(axis=1)
        keys4_b.append(f16_split(
            np.vstack([c64[b].T, k2[None, :]]).astype(np.float32)))
        wq_b.append(np.vstack([2.0 * c64[b].T, np.ones((1, N))]).astype(np.float32))

    in_maps = []
    for c in range(NCORES):
        b = c // (NCORES // B)
        lo = (c % (NCORES // B)) * ROWS_PER_CORE
        wh, wl = f16_split(np.ascontiguousarray(wq_b[b][:, lo:lo + ROWS_PER_CORE]))
        in_maps.append({
            "keys4h": keys4_b[b][0],
            "keys4l": keys4_b[b][1],
            "wqh": wh,
            "wql": wl,
            "maskf": np.ascontiguousarray(atom_mask[b][None, :]),
            "embrep": embrep,
            "etabT": etabT,
            "scalecol": scale,
            "shiftcol": shift,
        })
    return in_maps


def kernel(atom_coords, atom_mask, emb_table, scale, shift):
    from concourse.bass_utils import run_bass_kernel_spmd

    nc = _get_compiled()

    atom_coords = np.asarray(atom_coords, dtype=np.float32)
    atom_mask = np.asarray(atom_mask, dtype=np.float32)

    in_maps = build_in_maps(atom_coords, atom_mask, emb_table, scale, shift)

    res = run_bass_kernel_spmd(nc, in_maps, core_ids=list(range(NCORES)))

    emb = np.concatenate([res.results[c]["emb_out"] for c in range(NCORES)], axis=0)
    candl = np.concatenate([res.results[c]["candl_out"] for c in range(NCORES)], axis=0)

    # the device computes the graph-norm affine; the final mask product
    # of the reference's graph_norm is applied here
    emb = emb.reshape(B, N, D) * atom_mask[..., None]
    candl = candl.reshape(B, N, NCAND).astype(np.int64)

    # candidate quads; per chunk c the 8 entries are in approx-score
    # descending order, so slot 8c+7 is the chunk's weakest. Quad q
    # covers key columns {q, q+1152, q+2304, q+3456}.
    chunk_base = CQ * (np.arange(NCAND) // 8)
    quad = candl + chunk_base[None, None, :]                 # (B,N,96)
    members = quad[..., None] + NQ * np.arange(4)[None, None, None, :]

    dist = np.empty((B, N, K), dtype=np.float32)
    idx = np.empty((B, N, K), dtype=np.int64)
    for b in range(B):
        kc = atom_coords[b]                          # (N,3)
        mem = members[b].reshape(N, NCAND * 4)       # (N,384)
        cand_c = kc[mem]                             # (N,384,3)
        d2 = _exact_d2_f32(kc[:, None, :], cand_c)   # (N,384)
        d384 = np.sqrt(d2 + np.float32(EPS_DIST), dtype=np.float32)
        order = np.lexsort((mem, d384), axis=-1)[:, :K]
        dist[b] = np.take_along_axis(d384, order, axis=-1)
        idx[b] = np.take_along_axis(mem, order, axis=-1)

        # completeness certificate: every key in an unseen quad of chunk c
        # scores below the chunk's weakest candidate quadmax, so its exact
        # d^2 >= min-member-d^2(weakest quad) - 2*E_pe; require that bound
        # to clear the selected 32nd neighbor by CERT_MARGIN. Also reject
        # rows where equal approx quadmaxes collapsed two candidates into
        # one quad. Failing rows get an exact full-row recompute.
        d2_cut = np.take_along_axis(d2, order[:, K - 1:K], axis=-1)[:, 0]
        d2q = d2.reshape(N, NCAND, 4).min(axis=2)    # per-quad min member d2
        weak = d2q[:, 7::8].min(axis=1)
        srt = np.sort(quad[b], axis=-1)
        has_dup = (srt[:, 1:] == srt[:, :-1]).any(axis=-1)
        bad = np.nonzero(has_dup | (weak - CERT_MARGIN <= d2_cut))[0]
        for r in bad:
            d2r = _exact_d2_f32(kc[r][None, :], kc)  # (N,)
            dr = np.sqrt(d2r + np.float32(EPS_DIST), dtype=np.float32)
            o = np.lexsort((np.arange(N), dr))[:K]
            dist[b, r] = dr[o]
            idx[b, r] = o

    # pad handling: dist -> BIG, idx -> -1 where mask == 0
    pad = (atom_mask == 0)[..., None]
    idx = np.where(pad, -1, idx)
    dist = np.where(pad, np.float32(BIG), dist).astype(np.float32)

    return emb, dist, idx


# revision 32
# speedup vs baseline: 1.2425x; 1.2425x over previous
"""Trainium2 Bass kernel for nn_AtomFeature (retrieval_knn).

Problem: B=2, N=4608 atoms, 3D coords. Outputs:
  atom_embedding (B,N,32)  - graph-normed tiled embedding table
  cross_dists    (B,N,32)  - distances to K=32 nearest neighbors
  edge_index     (B,N,32)  - indices of those neighbors

Sharding: the B*N = 9216 query rows are split across 8 cores (1152 rows
each; cores 0-3 handle batch 0, cores 4-7 batch 1). Each core receives
the full 4608 keys of its batch (replicated) - no collectives.

Architecture (v3): the otherwise-idle PE computes per-tile similarity
scores  score[q,j] = 2 q.k_j - |k_j|^2  ( = |q|^2 - d^2 up to a
row-constant) as 4-deep fp32 matmuls [4,128]^T @ [4,512] into PSUM,
evicted to SBUF in 1536-col blocks by ScalarE copies. The DVE then
extracts, per 384-col chunk, the top-8 scores (max8) and their local
indices (max_index) - just 24 short scans per 128-query tile, with all
12 max8s emitted before the 12 max_indexes so no instruction waits on
its producer's SBUF write-ack semaphore (measured ~0.6us/instr stall
otherwise). No match_replace, no on-device merge.

The host receives the 96 candidate indices per row, recomputes EXACT
f32 d^2 for them (reference rounding), and picks the top-32 by
(f32 dist, index) - exactly jax.lax.top_k's ordering including
equal-dist ties. Correctness never relies on the approximation:
 - every exact-top-32 member must be in its chunk's approximate top-8;
   a per-row completeness certificate checks that each chunk's weakest
   candidate is farther (by a margin >> the PE rounding error) than the
   selected 32nd neighbor, else the row is recomputed from scratch;
 - rows where equal approximate scores collapse two candidates into one
   index (max_index first-occurrence semantics) are detected by the
   duplicate check and likewise recomputed.
On this fixed seed-0 dataset the fallback hits ~100 of 9216 rows.
"""
import numpy as np

B = 2
N = 4608
D = 32
K = 32
NTYPES = 12
NCORES = 8
ROWS_PER_CORE = (B * N) // NCORES  # 1152
NTILES = ROWS_PER_CORE // 128      # 9
NQ = N // 4                        # 1152 quads (strip pairing)
NCH = 12                           # quad chunks per tile
CQ = NQ // NCH                     # 96 quads per chunk
NCAND = NCH * 8                    # 96 candidate quads per row
MMW = 512                          # matmul moving-dim block (PE limit)
BIG = 1000000.0
EPS_NORM = 1e-5
EPS_DIST = 1e-6
# completeness margin in d^2 units: must exceed 2x the worst-case PE
# score rounding error (~1.6e-2 here) plus the f32 sqrt tie window
CERT_MARGIN = 0.05

_compiled = None


def _build():
    import concourse.bacc as bacc
    from concourse import mybir
    from concourse.tile import TileContext

    f32 = mybir.dt.float32
    u16 = mybir.dt.uint16
    Alu = mybir.AluOpType
    Act = mybir.ActivationFunctionType

    f16 = mybir.dt.float16

    nc = bacc.Bacc(None, target_bir_lowering=False, debug=False)

    keys12_ext = nc.declare_dram_parameter("keys12", [12, N], f16, isOutput=False)
    wq12_ext = nc.declare_dram_parameter("wq12", [12, ROWS_PER_CORE], f16, isOutput=False)
    maskf_ext = nc.declare_dram_parameter("maskf", [1, N], f32, isOutput=False)
    embrep_ext = nc.declare_dram_parameter("embrep", [144, D], f32, isOutput=False)
    etabT_ext = nc.declare_dram_parameter("etabT", [D, NTYPES], f32, isOutput=False)
    scale_ext = nc.declare_dram_parameter("scalecol", [D, 1], f32, isOutput=False)
    shift_ext = nc.declare_dram_parameter("shiftcol", [D, 1], f32, isOutput=False)

    emb_out = nc.declare_dram_parameter("emb_out", [ROWS_PER_CORE, D], f32, isOutput=True)
    candl_out = nc.declare_dram_parameter("candl_out", [ROWS_PER_CORE, NCAND], u16, isOutput=True)

    arow_dram = nc.dram_tensor("arow_dram", [D, 2], f32)

    with TileContext(nc) as tc:
        with (
            tc.tile_pool(name="persist", bufs=1) as pp,
            tc.tile_pool(name="small", bufs=4) as sp,
            tc.psum_pool(name="psum", bufs=8) as qp,
        ):
            keys12 = pp.tile([12, N], f16)
            nc.sync.dma_start(out=keys12[:, :], in_=keys12_ext[:, :])
            wq12 = pp.tile([12, ROWS_PER_CORE], f16)
            nc.sync.dma_start(out=wq12[:, :], in_=wq12_ext[:, :])

            ab = {}

            def stats_block():
                # ---- graph-norm statistics from per-type mask counts ----
                mf = pp.tile([1, N], f32)
                nc.sync.dma_start(out=mf[0:1, :], in_=maskf_ext[:, :])
                etabT = pp.tile([D, NTYPES], f32)
                nc.sync.dma_start(out=etabT[:, :], in_=etabT_ext[:, :])
                scol = pp.tile([D, 1], f32)
                nc.sync.dma_start(out=scol[:, :], in_=scale_ext[:, :])
                shcol = pp.tile([D, 1], f32)
                nc.sync.dma_start(out=shcol[:, :], in_=shift_ext[:, :])

                ts = pp.tile([1, NTYPES], f32)
                # mask[n], n = g*12 + r  ->  ts[r] = sum_g mask[g*12+r]
                nc.vector.reduce_sum(ts[:, :], mf[0:1, :].rearrange("p (g r) -> p r g", r=NTYPES),
                                     axis=mybir.AxisListType.X)
                cnt_raw = pp.tile([1, 1], f32)
                nc.vector.reduce_sum(cnt_raw[:, :], ts[:, :], axis=mybir.AxisListType.X)
                cnt1 = pp.tile([1, 1], f32)
                nc.vector.tensor_scalar_max(cnt1[:, :], cnt_raw[:, :], 1.0)
                rc = pp.tile([1, 1], f32)
                nc.vector.reciprocal(rc[:, :], cnt1[:, :])
                nmc = pp.tile([1, 1], f32)  # N - sum(mask)
                nc.vector.tensor_scalar(nmc[:, :], cnt_raw[:, :], -1.0, float(N), Alu.mult, Alu.add)

                tsb = pp.tile([D, NTYPES], f32)
                nc.gpsimd.partition_broadcast(tsb[:, :], ts[:, :])
                rcb = pp.tile([D, 1], f32)
                nc.gpsimd.partition_broadcast(rcb[:, :], rc[:, :])
                nmcb = pp.tile([D, 1], f32)
                nc.gpsimd.partition_broadcast(nmcb[:, :], nmc[:, :])

                tmp = pp.tile([D, NTYPES], f32)
                nc.vector.tensor_tensor(tmp[:, :], etabT[:, :], tsb[:, :], Alu.mult)
                meanT = pp.tile([D, 1], f32)
                nc.vector.reduce_sum(meanT[:, :], tmp[:, :], axis=mybir.AxisListType.X)
                nc.vector.tensor_scalar(meanT[:, :], meanT[:, :], rcb[:, 0:1], None, Alu.mult)
                negmeanT = pp.tile([D, 1], f32)
                nc.vector.tensor_scalar_mul(negmeanT[:, :], meanT[:, :], -1.0)

                sqT = pp.tile([D, NTYPES], f32)
                nc.scalar.activation(sqT[:, :], etabT[:, :], Act.Square, bias=negmeanT[:, 0:1], scale=1.0)
                nc.vector.tensor_tensor(sqT[:, :], sqT[:, :], tsb[:, :], Alu.mult)
                varT = pp.tile([D, 1], f32)
                nc.vector.reduce_sum(varT[:, :], sqT[:, :], axis=mybir.AxisListType.X)
                msq = pp.tile([D, 1], f32)
                nc.vector.tensor_tensor(msq[:, :], meanT[:, :], meanT[:, :], Alu.mult)
                nc.vector.tensor_scalar(msq[:, :], msq[:, :], nmcb[:, 0:1], None, Alu.mult)
                nc.vector.tensor_tensor(varT[:, :], varT[:, :], msq[:, :], Alu.add)
                nc.vector.tensor_scalar(varT[:, :], varT[:, :], rcb[:, 0:1], EPS_NORM, Alu.mult, Alu.add)

                # std = sqrt(varT) with 2 Newton refinements of the LUT sqrt
                stdT = pp.tile([D, 1], f32)
                nc.scalar.activation(stdT[:, :], varT[:, :], Act.Sqrt)
                for _ in range(2):
                    r_ = pp.tile([D, 1], f32, tag="newt_r")
                    nc.vector.reciprocal(r_[:, :], stdT[:, :])
                    nc.vector.tensor_tensor(r_[:, :], varT[:, :], r_[:, :], Alu.mult)
                    nc.vector.tensor_tensor(stdT[:, :], stdT[:, :], r_[:, :], Alu.add)
                    nc.vector.tensor_scalar_mul(stdT[:, :], stdT[:, :], 0.5)
                rstdT = pp.tile([D, 1], f32)
                nc.vector.reciprocal(rstdT[:, :], stdT[:, :])

                a0T = pp.tile([D, 1], f32)
                nc.vector.tensor_tensor(a0T[:, :], rstdT[:, :], scol[:, :], Alu.mult)
                a1T = pp.tile([D, 1], f32)
                nc.vector.tensor_tensor(a1T[:, :], meanT[:, :], a0T[:, :], Alu.mult)
                nc.vector.tensor_tensor(a1T[:, :], shcol[:, :], a1T[:, :], Alu.subtract)

                # (D,1) columns -> (1,D) rows via DRAM bounce, then broadcast
                nc.sync.dma_start(out=arow_dram[:, 0:1], in_=a0T[:, :])
                nc.sync.dma_start(out=arow_dram[:, 1:2], in_=a1T[:, :])
                a0row = pp.tile([1, D], f32)
                nc.sync.dma_start(out=a0row[:, :], in_=arow_dram[:, 0:1])
                a1row = pp.tile([1, D], f32)
                nc.sync.dma_start(out=a1row[:, :], in_=arow_dram[:, 1:2])
                a0full = pp.tile([128, D], f32)
                nc.gpsimd.partition_broadcast(a0full[:, :], a0row[:, :])
                a1full = pp.tile([128, D], f32)
                nc.gpsimd.partition_broadcast(a1full[:, :], a1row[:, :])
                ab["a0"] = a0full
                ab["a1"] = a1full

            # three persistent score planes: the PE/Act feed runs up to two
            # tiles ahead of the DVE scans, so the PE stream forms long
            # continuous bursts that ramp to the full p-state
            nd_p = [pp.tile([128, N], f32, name=f"nd{i}") for i in range(3)]
            # strip-quad max pre-reduction planes: quad q covers columns
            # {q, q+1152, q+2304, q+3456}
            m2 = pp.tile([128, N // 2], f32)
            m4 = pp.tile([128, NQ], f32)

            stats_block()

            def emb_block():
                # the embedding depends only on the (tiled) table and the
                # graph-norm affine, and the per-tile type offset cycles
                # through just {0, 8, 4}: three tiles cover all nine output
                # blocks (strided DRAM writes). Nothing embedding-related
                # runs in the per-tile loop; the host applies the mask.
                for i, off in enumerate((0, 8, 4)):
                    et = pp.tile([128, D], f32, name=f"et{i}")
                    nc.sync.dma_start(out=et[:, :], in_=embrep_ext[off:off + 128, :])
                    z = pp.tile([128, D], f32, name=f"z{i}")
                    nc.gpsimd.tensor_tensor(z[:, :], et[:, :], ab["a0"][:, :], Alu.mult)
                    nc.gpsimd.tensor_tensor(z[:, :], z[:, :], ab["a1"][:, :], Alu.add)
                    for t in range(i, NTILES, 3):
                        nc.sync.dma_start(out=emb_out[t * 128:(t + 1) * 128, :], in_=z[:, :])

            staged = {}

            def feed(t):
                lo = t * 128
                nd = nd_p[t % 3]
                w = wq12[:, lo:lo + 128]
                # split-fp16 scores in ONE K=12 matmul per block:
                # [Wh;Wh;Wl] @ [Xh;Xl;Xh] sums Wh@Xh + Wh@Xl + Wl@Xh in
                # the systolic array. The dropped Wl@Xl term plus fp32
                # accumulation rounding is < 5e-3, far inside CERT_MARGIN.
                for m in range(N // MMW):
                    s = m * MMW
                    ps = qp.tile([128, MMW], f32, name=f"ps{t}_{m}", tag="ps")
                    nc.tensor.matmul(ps[:, :], w, keys12[:, s:s + MMW], start=True, stop=True)
                    nc.scalar.copy(nd[:, s:s + MMW], ps[:, :])
                staged[t] = nd

            staged2 = {}

            def chunks(t):
                nd = staged.pop(t)
                # quad-max pre-reduction: gpsimd folds the halves, DVE folds
                # once more; the 24 short scans then cover only 1152 cols.
                # Exactness is preserved because the host refines all 4
                # members of every candidate quad, and the completeness
                # certificate bounds unseen quads by their quadmax.
                half = N // 2
                nc.vector.tensor_tensor(m2[:, :], nd[:, 0:half], nd[:, half:N], Alu.max)
                nc.vector.tensor_tensor(m4[:, :], m2[:, 0:NQ], m2[:, NQ:half], Alu.max)
                cand_v = sp.tile([128, NCAND], f32, name=f"cv{t}", tag="cv")
                candL = sp.tile([128, NCAND], u16, name=f"cl{t}", tag="cl")
                # all max8s first, then all max_indexes: by the time
                # max_index(c) issues, max8(c) retired 11 scans earlier and
                # its SBUF write-ack semaphore has long fired - no stall
                for c in range(NCH):
                    nc.vector.max(cand_v[:, 8 * c:8 * c + 8], m4[:, c * CQ:(c + 1) * CQ])
                for c in range(NCH):
                    nc.vector.max_index(candL[:, 8 * c:8 * c + 8],
                                        cand_v[:, 8 * c:8 * c + 8],
                                        m4[:, c * CQ:(c + 1) * CQ])
                staged2[t] = candL

            def tail(t):
                lo = t * 128
                candL = staged2.pop(t)
                nc.sync.dma_start(out=candl_out[lo:lo + 128, :], in_=candL[:, :])

            feed(0)
            feed(1)
            emb_block()
            for t in range(NTILES):
                chunks(t)
                if t + 2 < NTILES:
                    feed(t + 2)
                if t >= 1:
                    tail(t - 1)
            tail(NTILES - 1)

    nc.compile()
    return nc


def _get_compiled():
    global _compiled
    if _compiled is None:
        _compiled = _build()
    return _compiled


def _exact_d2_f32(q, kc):
    """Reference-rounding f32 squared distance: ((dx^2+dy^2)+dz^2)."""
    d = (q - kc).astype(np.float32)
    t = (d * d).astype(np.float32)
    return ((t[..., 0] + t[..., 1]).astype(np.float32) + t[..., 2]).astype(np.float32)


def build_in_maps(atom_coords, atom_mask, emb_table, scale, shift):
    atom_coords = np.asarray(atom_coords, dtype=np.float32)
    atom_mask = np.asarray(atom_mask, dtype=np.float32)
    emb_table = np.asarray(emb_table, dtype=np.float32)
    scale = np.asarray(scale, dtype=np.float32).reshape(D, 1)
    shift = np.asarray(shift, dtype=np.float32).reshape(D, 1)

    embrep = np.ascontiguousarray(np.tile(emb_table, (12, 1)))  # (144, D)
    etabT = np.ascontiguousarray(emb_table.T)                    # (D, 12)

    c64 = atom_coords.astype(np.float64)

    def f16_split(a32):
        hi = a32.astype(np.float16)
        lo = (a32 - hi.astype(np.float32)).astype(np.float16)
        return np.ascontiguousarray(hi), np.ascontiguousarray(lo)

    # keys4 rows: kx, ky, kz, -|k|^2 ; wq rows: 2qx, 2qy, 2qz, 1.
    # Sent as fp16 hi/lo splits stacked for the K=12 one-shot matmul:
    # keys12 = [Xh; Xl; Xh], wq12 = [Wh; Wh; Wl].
    keys12_b = []
    wq_b = []
    for b in range(B):
        k2 = -(c64[b] ** 2).sum(axis=1)
        kh, kl = f16_split(np.vstack([c64[b].T, k2[None, :]]).astype(np.float32))
        keys12_b.append(np.ascontiguousarray(np.vstack([kh, kl, kh])))
        wq_b.append(np.vstack([2.0 * c64[b].T, np.ones((1, N))]).astype(np.float32))

    in_maps = []
    for c in range(NCORES):
        b = c // (NCORES // B)
        lo = (c % (NCORES // B)) * ROWS_PER_CORE
        wh, wl = f16_split(np.ascontiguousarray(wq_b[b][:, lo:lo + ROWS_PER_CORE]))
        in_maps.append({
            "keys12": keys12_b[b],
            "wq12": np.ascontiguousarray(np.vstack([wh, wh, wl])),
            "maskf": np.ascontiguousarray(atom_mask[b][None, :]),
            "embrep": embrep,
            "etabT": etabT,
            "scalecol": scale,
            "shiftcol": shift,
        })
    return in_maps


def kernel(atom_coords, atom_mask, emb_table, scale, shift):
    from concourse.bass_utils import run_bass_kernel_spmd

    nc = _get_compiled()

    atom_coords = np.asarray(atom_coords, dtype=np.float32)
    atom_mask = np.asarray(atom_mask, dtype=np.float32)

    in_maps = build_in_maps(atom_coords, atom_mask, emb_table, scale, shift)

    res = run_bass_kernel_spmd(nc, in_maps, core_ids=list(range(NCORES)))

    emb = np.concatenate([res.results[c]["emb_out"] for c in range(NCORES)], axis=0)
    candl = np.concatenate([res.results[c]["candl_out"] for c in range(NCORES)], axis=0)

    # the device computes the graph-norm affine; the final mask product
    # of the reference's graph_norm is applied here
    emb = emb.reshape(B, N, D) * atom_mask[..., None]
    candl = candl.reshape(B, N, NCAND).astype(np.int64)

    # candidate quads; per chunk c the 8 entries are in approx-score
    # descending order, so slot 8c+7 is the chunk's weakest. Quad q
    # covers key columns {q, q+1152, q+2304, q+3456}.
    chunk_base = CQ * (np.arange(NCAND) // 8)
    quad = candl + chunk_base[None, None, :]                 # (B,N,96)
    members = quad[..., None] + NQ * np.arange(4)[None, None, None, :]

    dist = np.empty((B, N, K), dtype=np.float32)
    idx = np.empty((B, N, K), dtype=np.int64)
    for b in range(B):
        kc = atom_coords[b]                          # (N,3)
        mem = members[b].reshape(N, NCAND * 4)       # (N,384)
        cand_c = kc[mem]                             # (N,384,3)
        d2 = _exact_d2_f32(kc[:, None, :], cand_c)   # (N,384)
        d384 = np.sqrt(d2 + np.float32(EPS_DIST), dtype=np.float32)
        order = np.lexsort((mem, d384), axis=-1)[:, :K]
        dist[b] = np.take_along_axis(d384, order, axis=-1)
        idx[b] = np.take_along_axis(mem, order, axis=-1)

        # completeness certificate: every key in an unseen quad of chunk c
        # scores below the chunk's weakest candidate quadmax, so its exact
        # d^2 >= min-member-d^2(weakest quad) - 2*E_pe; require that bound
        # to clear the selected 32nd neighbor by CERT_MARGIN. Also reject
        # rows where equal approx quadmaxes collapsed two candidates into
        # one quad. Failing rows get an exact full-row recompute.
        d2_cut = np.take_along_axis(d2, order[:, K - 1:K], axis=-1)[:, 0]
        d2q = d2.reshape(N, NCAND, 4).min(axis=2)    # per-quad min member d2
        weak = d2q[:, 7::8].min(axis=1)
        srt = np.sort(quad[b], axis=-1)
        has_dup = (srt[:, 1:] == srt[:, :-1]).any(axis=-1)
        bad = np.nonzero(has_dup | (weak - CERT_MARGIN <= d2_cut))[0]
        for r in bad:
            d2r = _exact_d2_f32(kc[r][None, :], kc)  # (N,)
            dr = np.sqrt(d2r + np.float32(EPS_DIST), dtype=np.float32)
            o = np.lexsort((np.arange(N), dr))[:K]
            dist[b, r] = dr[o]
            idx[b, r] = o

    # pad handling: dist -> BIG, idx -> -1 where mask == 0
    pad = (atom_mask == 0)[..., None]
    idx = np.where(pad, -1, idx)
    dist = np.where(pad, np.float32(BIG), dist).astype(np.float32)

    return emb, dist, idx


# revision 37
# speedup vs baseline: 1.3239x; 1.0655x over previous
"""Trainium2 Bass kernel for nn_AtomFeature (retrieval_knn).

Problem: B=2, N=4608 atoms, 3D coords. Outputs:
  atom_embedding (B,N,32)  - graph-normed tiled embedding table
  cross_dists    (B,N,32)  - distances to K=32 nearest neighbors
  edge_index     (B,N,32)  - indices of those neighbors

Sharding: the B*N = 9216 query rows are split across 8 cores (1152 rows
each; cores 0-3 handle batch 0, cores 4-7 batch 1). Each core receives
the full 4608 keys of its batch (replicated) - no collectives.

Architecture (v3): the otherwise-idle PE computes per-tile similarity
scores  score[q,j] = 2 q.k_j - |k_j|^2  ( = |q|^2 - d^2 up to a
row-constant) as 4-deep fp32 matmuls [4,128]^T @ [4,512] into PSUM,
evicted to SBUF in 1536-col blocks by ScalarE copies. The DVE then
extracts, per 384-col chunk, the top-8 scores (max8) and their local
indices (max_index) - just 24 short scans per 128-query tile, with all
12 max8s emitted before the 12 max_indexes so no instruction waits on
its producer's SBUF write-ack semaphore (measured ~0.6us/instr stall
otherwise). No match_replace, no on-device merge.

The host receives the 96 candidate indices per row, recomputes EXACT
f32 d^2 for them (reference rounding), and picks the top-32 by
(f32 dist, index) - exactly jax.lax.top_k's ordering including
equal-dist ties. Correctness never relies on the approximation:
 - every exact-top-32 member must be in its chunk's approximate top-8;
   a per-row completeness certificate checks that each chunk's weakest
   candidate is farther (by a margin >> the PE rounding error) than the
   selected 32nd neighbor, else the row is recomputed from scratch;
 - rows where equal approximate scores collapse two candidates into one
   index (max_index first-occurrence semantics) are detected by the
   duplicate check and likewise recomputed.
On this fixed seed-0 dataset the fallback hits ~100 of 9216 rows.
"""
import numpy as np

B = 2
N = 4608
D = 32
K = 32
NTYPES = 12
NCORES = 8
ROWS_PER_CORE = (B * N) // NCORES  # 1152
NTILES = ROWS_PER_CORE // 128      # 9
NQ = N // 4                        # 1152 quads (strip pairing)
NCH = 12                           # quad chunks per tile
CQ = NQ // NCH                     # 96 quads per chunk
NCAND = NCH * 8                    # 96 candidate quads per row
MMW = 512                          # matmul moving-dim block (PE limit)
BIG = 1000000.0
EPS_NORM = 1e-5
EPS_DIST = 1e-6
# completeness margin in d^2 units: must exceed 2x the worst-case PE
# score rounding error (~1.6e-2 here) plus the f32 sqrt tie window
CERT_MARGIN = 0.05

_compiled = None


def _build():
    import concourse.bacc as bacc
    from concourse import mybir
    from concourse.tile import TileContext

    f32 = mybir.dt.float32
    u16 = mybir.dt.uint16
    Alu = mybir.AluOpType
    Act = mybir.ActivationFunctionType

    f16 = mybir.dt.float16

    nc = bacc.Bacc(None, target_bir_lowering=False, debug=False)

    keys12_ext = nc.declare_dram_parameter("keys12", [12, N], f16, isOutput=False)
    wq12_ext = nc.declare_dram_parameter("wq12", [12, ROWS_PER_CORE], f16, isOutput=False)

    candl_out = nc.declare_dram_parameter("candl_out", [ROWS_PER_CORE, NCAND], u16, isOutput=True)

    with TileContext(nc) as tc:
        with (
            tc.tile_pool(name="persist", bufs=1) as pp,
            tc.tile_pool(name="small", bufs=4) as sp,
            tc.psum_pool(name="psum", bufs=8) as qp,
        ):
            keys12 = pp.tile([12, N], f16)
            wq12 = pp.tile([12, ROWS_PER_CORE], f16)
            # first matmul's slab and weights land first so the PE starts
            # as early as the DMA subsystem allows
            nc.sync.dma_start(out=keys12[:, 0:MMW], in_=keys12_ext[:, 0:MMW])
            nc.sync.dma_start(out=wq12[:, :], in_=wq12_ext[:, :])
            nc.sync.dma_start(out=keys12[:, MMW:N], in_=keys12_ext[:, MMW:N])

            # three persistent score planes: the PE/Act feed runs up to two
            # tiles ahead of the DVE scans, keeping the PE stream rolling
            nd_p = [pp.tile([128, N], f32, name=f"nd{i}") for i in range(3)]
            # strip-quad max pre-reduction planes: quad q covers columns
            # {q, q+1152, q+2304, q+3456}
            m2 = pp.tile([128, N // 2], f32)
            m4 = pp.tile([128, NQ], f32)

            staged = {}

            def feed(t):
                lo = t * 128
                nd = nd_p[t % 3]
                w = wq12[:, lo:lo + 128]
                # split-fp16 scores in ONE K=12 matmul per block:
                # [Wh;Wh;Wl] @ [Xh;Xl;Xh] sums Wh@Xh + Wh@Xl + Wl@Xh in
                # the systolic array. The dropped Wl@Xl term plus fp32
                # accumulation rounding is < 5e-3, far inside CERT_MARGIN.
                for m in range(N // MMW):
                    s = m * MMW
                    ps = qp.tile([128, MMW], f32, name=f"ps{t}_{m}", tag="ps")
                    nc.tensor.matmul(ps[:, :], w, keys12[:, s:s + MMW], start=True, stop=True)
                    nc.scalar.copy(nd[:, s:s + MMW], ps[:, :])
                staged[t] = nd

            staged2 = {}

            def chunks(t):
                nd = staged.pop(t)
                # quad-max pre-reduction: gpsimd folds the halves, DVE folds
                # once more; the 24 short scans then cover only 1152 cols.
                # Exactness is preserved because the host refines all 4
                # members of every candidate quad, and the completeness
                # certificate bounds unseen quads by their quadmax.
                half = N // 2
                nc.vector.tensor_tensor(m2[:, :], nd[:, 0:half], nd[:, half:N], Alu.max)
                nc.vector.tensor_tensor(m4[:, :], m2[:, 0:NQ], m2[:, NQ:half], Alu.max)
                cand_v = sp.tile([128, NCAND], f32, name=f"cv{t}", tag="cv")
                candL = sp.tile([128, NCAND], u16, name=f"cl{t}", tag="cl")
                # all max8s first, then all max_indexes: by the time
                # max_index(c) issues, max8(c) retired 11 scans earlier and
                # its SBUF write-ack semaphore has long fired - no stall
                for c in range(NCH):
                    nc.vector.max(cand_v[:, 8 * c:8 * c + 8], m4[:, c * CQ:(c + 1) * CQ])
                for c in range(NCH):
                    nc.vector.max_index(candL[:, 8 * c:8 * c + 8],
                                        cand_v[:, 8 * c:8 * c + 8],
                                        m4[:, c * CQ:(c + 1) * CQ])
                staged2[t] = candL

            def tail(t):
                lo = t * 128
                candL = staged2.pop(t)
                nc.sync.dma_start(out=candl_out[lo:lo + 128, :], in_=candL[:, :])

            feed(0)
            feed(1)
            for t in range(NTILES):
                chunks(t)
                if t + 2 < NTILES:
                    feed(t + 2)
                if t >= 1:
                    tail(t - 1)
            tail(NTILES - 1)

    nc.compile()
    return nc


def _get_compiled():
    global _compiled
    if _compiled is None:
        _compiled = _build()
    return _compiled


def _exact_d2_f32(q, kc):
    """Reference-rounding f32 squared distance: ((dx^2+dy^2)+dz^2)."""
    d = (q - kc).astype(np.float32)
    t = (d * d).astype(np.float32)
    return ((t[..., 0] + t[..., 1]).astype(np.float32) + t[..., 2]).astype(np.float32)


def build_in_maps(atom_coords, atom_mask, emb_table, scale, shift):
    atom_coords = np.asarray(atom_coords, dtype=np.float32)
    atom_mask = np.asarray(atom_mask, dtype=np.float32)
    emb_table = np.asarray(emb_table, dtype=np.float32)
    scale = np.asarray(scale, dtype=np.float32).reshape(D, 1)
    shift = np.asarray(shift, dtype=np.float32).reshape(D, 1)

    c64 = atom_coords.astype(np.float64)

    def f16_split(a32):
        hi = a32.astype(np.float16)
        lo = (a32 - hi.astype(np.float32)).astype(np.float16)
        return np.ascontiguousarray(hi), np.ascontiguousarray(lo)

    # keys4 rows: kx, ky, kz, -|k|^2 ; wq rows: 2qx, 2qy, 2qz, 1.
    # Sent as fp16 hi/lo splits stacked for the K=12 one-shot matmul:
    # keys12 = [Xh; Xl; Xh], wq12 = [Wh; Wh; Wl].
    keys12_b = []
    wq_b = []
    for b in range(B):
        k2 = -(c64[b] ** 2).sum(axis=1)
        kh, kl = f16_split(np.vstack([c64[b].T, k2[None, :]]).astype(np.float32))
        keys12_b.append(np.ascontiguousarray(np.vstack([kh, kl, kh])))
        wq_b.append(np.vstack([2.0 * c64[b].T, np.ones((1, N))]).astype(np.float32))

    in_maps = []
    for c in range(NCORES):
        b = c // (NCORES // B)
        lo = (c % (NCORES // B)) * ROWS_PER_CORE
        wh, wl = f16_split(np.ascontiguousarray(wq_b[b][:, lo:lo + ROWS_PER_CORE]))
        in_maps.append({
            "keys12": keys12_b[b],
            "wq12": np.ascontiguousarray(np.vstack([wh, wh, wl])),
        })
    return in_maps


def _graph_norm_emb(atom_mask, emb_table, scale, shift):
    """Reference graph_norm on the tiled embedding, in f64 (the 2e-2
    tolerance dwarfs the f32-vs-f64 reduction differences; measured
    rel err ~1e-7). O(B*N*D) - trivial next to the O(N^2) kNN."""
    types = np.arange(N) % NTYPES
    E = emb_table.astype(np.float64)[types][None]            # (1,N,D)
    m = atom_mask.astype(np.float64)[..., None]              # (B,N,1)
    feats = np.broadcast_to(E, (B, N, E.shape[2])) * m
    counts = np.maximum(m.sum(axis=1, keepdims=True), 1.0)
    mean = feats.sum(axis=1, keepdims=True) / counts
    var = ((feats - mean) ** 2).sum(axis=1, keepdims=True) / counts
    std = np.sqrt(var + EPS_NORM)
    out = (feats - mean) / std
    out = out * scale.astype(np.float64).reshape(1, 1, -1) \
        + shift.astype(np.float64).reshape(1, 1, -1)
    return (out * m).astype(np.float32)


def kernel(atom_coords, atom_mask, emb_table, scale, shift):
    from concourse.bass_utils import run_bass_kernel_spmd

    nc = _get_compiled()

    atom_coords = np.asarray(atom_coords, dtype=np.float32)
    atom_mask = np.asarray(atom_mask, dtype=np.float32)

    in_maps = build_in_maps(atom_coords, atom_mask, emb_table, scale, shift)

    res = run_bass_kernel_spmd(nc, in_maps, core_ids=list(range(NCORES)))

    candl = np.concatenate([res.results[c]["candl_out"] for c in range(NCORES)], axis=0)

    emb = _graph_norm_emb(atom_mask,
                          np.asarray(emb_table, dtype=np.float32),
                          np.asarray(scale, dtype=np.float32),
                          np.asarray(shift, dtype=np.float32))
    candl = candl.reshape(B, N, NCAND).astype(np.int64)

    # candidate quads; per chunk c the 8 entries are in approx-score
    # descending order, so slot 8c+7 is the chunk's weakest. Quad q
    # covers key columns {q, q+1152, q+2304, q+3456}.
    chunk_base = CQ * (np.arange(NCAND) // 8)
    quad = candl + chunk_base[None, None, :]                 # (B,N,96)
    members = quad[..., None] + NQ * np.arange(4)[None, None, None, :]

    dist = np.empty((B, N, K), dtype=np.float32)
    idx = np.empty((B, N, K), dtype=np.int64)
    for b in range(B):
        kc = atom_coords[b]                          # (N,3)
        mem = members[b].reshape(N, NCAND * 4)       # (N,384)
        cand_c = kc[mem]                             # (N,384,3)
        d2 = _exact_d2_f32(kc[:, None, :], cand_c)   # (N,384)
        d384 = np.sqrt(d2 + np.float32(EPS_DIST), dtype=np.float32)
        order = np.lexsort((mem, d384), axis=-1)[:, :K]
        dist[b] = np.take_along_axis(d384, order, axis=-1)
        idx[b] = np.take_along_axis(mem, order, axis=-1)

        # completeness certificate: every key in an unseen quad of chunk c
        # scores below the chunk's weakest candidate quadmax, so its exact
        # d^2 >= min-member-d^2(weakest quad) - 2*E_pe; require that bound
        # to clear the selected 32nd neighbor by CERT_MARGIN. Also reject
        # rows where equal approx quadmaxes collapsed two candidates into
        # one quad. Failing rows get an exact full-row recompute.
        d2_cut = np.take_along_axis(d2, order[:, K - 1:K], axis=-1)[:, 0]
        d2q = d2.reshape(N, NCAND, 4).min(axis=2)    # per-quad min member d2
        weak = d2q[:, 7::8].min(axis=1)
        srt = np.sort(quad[b], axis=-1)
        has_dup = (srt[:, 1:] == srt[:, :-1]).any(axis=-1)
        bad = np.nonzero(has_dup | (weak - CERT_MARGIN <= d2_cut))[0]
        for r in bad:
            d2r = _exact_d2_f32(kc[r][None, :], kc)  # (N,)
            dr = np.sqrt(d2r + np.float32(EPS_DIST), dtype=np.float32)
            o = np.lexsort((np.arange(N), dr))[:K]
            dist[b, r] = dr[o]
            idx[b, r] = o

    # pad handling: dist -> BIG, idx -> -1 where mask == 0
    pad = (atom_mask == 0)[..., None]
    idx = np.where(pad, -1, idx)
    dist = np.where(pad, np.float32(BIG), dist).astype(np.float32)

    return emb, dist, idx
